# revision 1
# baseline (speedup 1.0000x reference)
"""Trainium2 Bass kernel for nn_AttentionLayer (B=4, S=2048, H=16, DH=64).

Sharding: 8 cores = 4 batches x 2 head-groups (8 heads each). Each core
computes full attention for its (batch, head-group) shard; no cross-core
communication. The host pre-transposes/casts inputs, and post-normalizes
(softmax denominator division), transposes back, and adds the value bias.

Device dataflow per core (all matmuls bf16, PSUM f32):
  qlT[d',s] = wq_sl.T-contract  (lhsT=wq tiles [d,128], rhs=qT [d,s])
  klT[d',s] likewise; vl[j,dh'] natural (lhsT=vT tiles [d,j], rhs=wv)
  scoresT[j,i] = sum_dh klT[dh,j]*qlT[dh,i]   (K=64, head pairs row-packed)
  E = exp(0.125*scoresT)  (ACT, PSUM->SBUF bf16)
  E *= maskT              (DVE, multiplicative mask == additive -10000)
  ctxUT[dh,i] += vl_aug[j,dh].T @ E[j,i]  (vl_aug has a ones column ->
                                           row 64 = softmax denominator)
Output: [520, 2048] f32 = 8 heads x (64 ctxUT rows) + 8 den rows.
"""

import numpy as np
import ml_dtypes

import concourse.bass as bass
import concourse.mybir as mybir
import concourse.tile as tile
from concourse import bacc
from concourse.bass_utils import run_bass_kernel_spmd

BF16 = mybir.dt.bfloat16
F32 = mybir.dt.float32

S = 2048      # sequence length
D = 1024      # model dim
DL = 512      # local d' (8 heads x 64)
DH = 64       # head dim
HL = 8        # local heads
KT = 8        # k-tiles over D
MT = 4        # m-tiles over DL (128 each)
SB = 4        # s blocks of 512
JT = 16       # j tiles of 128
IB = 4        # i blocks of 512

_GRAPH = None


def build_graph():
    nc = bacc.Bacc("TRN2", target_bir_lowering=False, debug=False)

    qT = nc.dram_tensor("qT", [D, S], BF16, kind="ExternalInput").ap()
    kT = nc.dram_tensor("kT", [D, S], BF16, kind="ExternalInput").ap()
    vT = nc.dram_tensor("vT", [D, S], BF16, kind="ExternalInput").ap()
    maskT = nc.dram_tensor("maskT", [S, S], BF16, kind="ExternalInput").ap()
    wq = nc.dram_tensor("wq", [D, DL], BF16, kind="ExternalInput").ap()
    wk = nc.dram_tensor("wk", [D, DL], BF16, kind="ExternalInput").ap()
    wv = nc.dram_tensor("wv", [D, DL], BF16, kind="ExternalInput").ap()
    bq = nc.dram_tensor("bq", [DL], F32, kind="ExternalInput").ap()
    bk = nc.dram_tensor("bk", [DL], F32, kind="ExternalInput").ap()
    out = nc.dram_tensor("out", [DL + HL, S], F32, kind="ExternalOutput").ap()

    with tile.TileContext(nc) as tc:
        _build_body(tc, nc, qT, kT, vT, maskT, wq, wk, wv, bq, bk, out)

    nc.compile()
    return nc


def _build_body(tc, nc, qT, kT, vT, maskT, wq, wk, wv, bq, bk, out):
    from contextlib import ExitStack

    with ExitStack() as ctx:
        const = ctx.enter_context(tc.tile_pool(name="const", bufs=1))
        acts = ctx.enter_context(tc.tile_pool(name="acts", bufs=1))
        qk_pool = ctx.enter_context(tc.tile_pool(name="qk", bufs=3))
        e_pool = ctx.enter_context(tc.tile_pool(name="epool", bufs=8))
        m_pool = ctx.enter_context(tc.tile_pool(name="mpool", bufs=12))
        o_pool = ctx.enter_context(tc.tile_pool(name="opool", bufs=6))

        # ---- weights / biases / persistent activations ----
        wq_sb = const.tile([128, KT, DL], BF16)
        wk_sb = const.tile([128, KT, DL], BF16)
        wv_sb = const.tile([128, KT, DL], BF16)
        nc.sync.dma_start(out=wq_sb[:], in_=wq.rearrange("(kt p) n -> p kt n", p=128))
        nc.scalar.dma_start(out=wk_sb[:], in_=wk.rearrange("(kt p) n -> p kt n", p=128))
        nc.sync.dma_start(out=wv_sb[:], in_=wv.rearrange("(kt p) n -> p kt n", p=128))
        bq_sb = const.tile([128, MT], F32)
        bk_sb = const.tile([128, MT], F32)
        nc.sync.dma_start(out=bq_sb[:], in_=bq.rearrange("(m p) -> p m", p=128))
        nc.sync.dma_start(out=bk_sb[:], in_=bk.rearrange("(m p) -> p m", p=128))
        zero_b = const.tile([128, 1], F32)
        nc.vector.memset(zero_b[:], 0.0)

        qlT_sb = acts.tile([128, MT, S], BF16)   # [d' partition, m-tile, s]
        klT_sb = acts.tile([128, MT, S], BF16)
        vl_sb = acts.tile([128, JT, HL, DH + 1], BF16)  # per j-tile, per head, +ones
        nc.vector.memset(vl_sb[:, :, :, DH], 1.0)

        # ---- phase 1: projections ----
        with tc.tile_pool(name="ppsum", bufs=2, space="PSUM") as ppsum:
            for sb in range(SB):
                ssl = slice(sb * 512, (sb + 1) * 512)
                qt = qk_pool.tile([128, KT, 512], BF16, tag="qt")
                kt_ = qk_pool.tile([128, KT, 512], BF16, tag="kt")
                vt = qk_pool.tile([128, KT, 512], BF16, tag="vt")
                nc.sync.dma_start(
                    out=qt[:], in_=qT[:, ssl].rearrange("(kt p) n -> p kt n", p=128))
                nc.scalar.dma_start(
                    out=kt_[:], in_=kT[:, ssl].rearrange("(kt p) n -> p kt n", p=128))
                nc.scalar.dma_start(
                    out=vt[:], in_=vT[:, ssl].rearrange("(kt p) n -> p kt n", p=128))
                for m in range(MT):
                    msl = slice(m * 128, (m + 1) * 128)
                    psq = ppsum.tile([128, 512], F32, tag="pq")
                    psk = ppsum.tile([128, 512], F32, tag="pk")
                    for kk in range(KT):
                        nc.tensor.matmul(
                            psq[:], wq_sb[:, kk, msl], qt[:, kk, :],
                            start=(kk == 0), stop=(kk == KT - 1))
                    for kk in range(KT):
                        nc.tensor.matmul(
                            psk[:], wk_sb[:, kk, msl], kt_[:, kk, :],
                            start=(kk == 0), stop=(kk == KT - 1))
                    nc.vector.tensor_scalar_add(
                        qlT_sb[:, m, ssl], psq[:], bq_sb[:, m:m + 1])
                    nc.vector.tensor_scalar_add(
                        klT_sb[:, m, ssl], psk[:], bk_sb[:, m:m + 1])
                for jj in range(MT):
                    jt = sb * 4 + jj
                    jsl = slice(jj * 128, (jj + 1) * 128)
                    psv = ppsum.tile([128, 512], F32, tag="pv")
                    for kk in range(KT):
                        nc.tensor.matmul(
                            psv[:], vt[:, kk, jsl], wv_sb[:, kk, :],
                            start=(kk == 0), stop=(kk == KT - 1))
                    nc.vector.tensor_copy(
                        vl_sb[:, jt, :, 0:DH],
                        psv[:].rearrange("p (h d) -> p h d", h=HL))

        # ---- phase 2: attention ----
        with (
            tc.tile_pool(name="spsum", bufs=2, space="PSUM") as spsum,
            tc.tile_pool(name="cpsum", bufs=2, space="PSUM") as cpsum,
        ):
            for hp in range(4):
                h0, h1 = 2 * hp, 2 * hp + 1
                for ib in range(IB):
                    isl = slice(ib * 512, (ib + 1) * 512)
                    ctx0 = cpsum.tile([DH + 1, 512], F32, tag="c0")
                    ctx1 = cpsum.tile([DH + 1, 512], F32, tag="c1")
                    for jt in range(JT):
                        jsl = slice(jt * 128, (jt + 1) * 128)
                        msk = m_pool.tile([128, 512], BF16, tag="msk")
                        nc.sync.dma_start(out=msk[:], in_=maskT[jsl, isl])
                        sc = spsum.tile([128, 1024], F32, tag="sc")
                        nc.tensor.matmul(
                            sc[:, 0:512],
                            klT_sb[0:64, hp, jsl], qlT_sb[0:64, hp, isl],
                            start=True, stop=True)
                        nc.tensor.matmul(
                            sc[:, 512:1024],
                            klT_sb[64:128, hp, jsl], qlT_sb[64:128, hp, isl],
                            start=True, stop=True)
                        E = e_pool.tile([128, 1024], BF16, tag="E")
                        nc.scalar.activation(
                            E[:], sc[:], mybir.ActivationFunctionType.Exp,
                            bias=zero_b[:], scale=0.125)
                        ev = E[:].rearrange("p (o n) -> p o n", o=2)
                        mb = msk[:].rearrange("p (o n) -> p o n", o=1)
                        mb = mb.broadcast_to([128, 2, 512])
                        nc.vector.tensor_tensor(
                            ev, ev, mb, mybir.AluOpType.mult)
                        nc.tensor.matmul(
                            ctx0[:], vl_sb[:, jt, h0, :], E[:, 0:512],
                            start=(jt == 0), stop=(jt == JT - 1))
                        nc.tensor.matmul(
                            ctx1[:], vl_sb[:, jt, h1, :], E[:, 512:1024],
                            start=(jt == 0), stop=(jt == JT - 1))
                    o0 = o_pool.tile([DH + 1, 512], F32, tag="o")
                    o1 = o_pool.tile([DH + 1, 512], F32, tag="o")
                    nc.vector.tensor_copy(o0[:], ctx0[:])
                    nc.vector.tensor_copy(o1[:], ctx1[:])
                    nc.sync.dma_start(
                        out=out[h0 * DH:(h0 + 1) * DH, isl], in_=o0[0:DH, :])
                    nc.sync.dma_start(
                        out=out[DL + h0:DL + h0 + 1, isl], in_=o0[DH:DH + 1, :])
                    nc.sync.dma_start(
                        out=out[h1 * DH:(h1 + 1) * DH, isl], in_=o1[0:DH, :])
                    nc.sync.dma_start(
                        out=out[DL + h1:DL + h1 + 1, isl], in_=o1[DH:DH + 1, :])


def _get_graph():
    global _GRAPH
    if _GRAPH is None:
        _GRAPH = build_graph()
    return _GRAPH


def make_in_maps(q, k, v, attention_mask, wq_kernel, wq_bias, wk_kernel,
                 wk_bias, wv_kernel, wv_bias):
    bf = ml_dtypes.bfloat16
    in_maps = []
    for c in range(8):
        b, hg = divmod(c, 2)
        sl = slice(hg * DL, (hg + 1) * DL)
        in_maps.append({
            "qT": np.asarray(q[b].T, dtype=bf),
            "kT": np.asarray(k[b].T, dtype=bf),
            "vT": np.asarray(v[b].T, dtype=bf),
            "maskT": np.asarray(attention_mask[b].T, dtype=bf),
            "wq": np.asarray(wq_kernel[:, sl], dtype=bf),
            "wk": np.asarray(wk_kernel[:, sl], dtype=bf),
            "wv": np.asarray(wv_kernel[:, sl], dtype=bf),
            "bq": np.asarray(wq_bias[sl], dtype=np.float32),
            "bk": np.asarray(wk_bias[sl], dtype=np.float32),
        })
    return in_maps


def assemble_output(results, wv_bias):
    B = 4
    out_full = np.empty((B, S, D), dtype=np.float32)
    for c in range(8):
        b, hg = divmod(c, 2)
        o = results[c]["out"]                      # [520, 2048]
        ctxUT = o[:DL].reshape(HL, DH, S)
        den = o[DL:DL + HL]                        # [8, 2048]
        ctxn = ctxUT / den[:, None, :]
        out_full[b, :, hg * DL:(hg + 1) * DL] = (
            ctxn.transpose(2, 0, 1).reshape(S, DL))
    out_full += np.asarray(wv_bias, dtype=np.float32)[None, None, :]
    return out_full


def kernel(q, k, v, attention_mask, wq_kernel, wq_bias, wk_kernel, wk_bias,
           wv_kernel, wv_bias):
    nc = _get_graph()
    in_maps = make_in_maps(q, k, v, attention_mask, wq_kernel, wq_bias,
                           wk_kernel, wk_bias, wv_kernel, wv_bias)
    res = run_bass_kernel_spmd(nc, in_maps, core_ids=list(range(8)))
    return assemble_output(res.results, wv_bias)



# revision 3
# speedup vs baseline: 1.0522x; 1.0522x over previous
"""Trainium2 Bass kernel for nn_AttentionLayer (B=4, S=2048, H=16, DH=64).

Sharding: 8 cores = 4 batches x 2 head-groups (8 heads each). Each core
computes full attention for its (batch, head-group) shard; no cross-core
communication. The host pre-transposes/casts inputs, and post-normalizes
(softmax denominator division), transposes back, and adds the value bias.

v2: single fused schedule. The QKV projections are interleaved into the
attention loop as tensor-engine filler so the scalar engine (exp, the
bottleneck at ~1.15us per [128,1024] ACTIVATE) starts after a short
lead-in and stays saturated. Context matmuls are deferred LAG iterations
behind the scores/exp pipeline so V-projection work early on does not
stall the ACT feed. Mask DMA stays on the sync HWDGE queue; bulk input
chunks and output stores ride the gpsimd SWDGE queue.

Device dataflow per core (all matmuls bf16, PSUM f32):
  qlT[d',s] = wq_sl.T-contract  (lhsT=wq tiles [d,128], rhs=qT [d,s])
  klT[d',s] likewise; vl[j,dh'] natural (lhsT=vT tiles [d,j], rhs=wv)
  scoresT[j,i] = sum_dh klT[dh,j]*qlT[dh,i]   (K=64, head pairs row-packed,
                                               concurrent via PE row tiling)
  E = exp(0.125*scoresT)  (ACT, PSUM->SBUF bf16)
  E *= maskT              (DVE, multiplicative mask == additive -10000)
  ctxUT[dh,i] += vl_aug[j,dh].T @ E[j,i]  (vl_aug has a ones column ->
                                           row 64 = softmax denominator)
Output: [520, 2048] f32 = 8 heads x (64 ctx rows + 1 den row).
"""

import numpy as np
import ml_dtypes

import concourse.bass as bass
import concourse.mybir as mybir
import concourse.tile as tile
from concourse import bacc
from concourse.bass_utils import run_bass_kernel_spmd

BF16 = mybir.dt.bfloat16
F32 = mybir.dt.float32

S = 2048      # sequence length
D = 1024      # model dim
DL = 512      # local d' (8 heads x 64)
DH = 64       # head dim
HL = 8        # local heads
KT = 8        # k-tiles over D
MT = 4        # m-tiles over DL (128 each)
SB = 4        # s blocks of 512
JT = 16       # j tiles of 128
IB = 4        # i blocks of 512
LAG = 4       # ctx-matmul deferral (iterations)

_GRAPH = None


def build_graph():
    nc = bacc.Bacc("TRN2", target_bir_lowering=False, debug=False)

    qT = nc.dram_tensor("qT", [D, S], BF16, kind="ExternalInput").ap()
    kT = nc.dram_tensor("kT", [D, S], BF16, kind="ExternalInput").ap()
    vT = nc.dram_tensor("vT", [D, S], BF16, kind="ExternalInput").ap()
    maskT = nc.dram_tensor("maskT", [S, S], BF16, kind="ExternalInput").ap()
    wq = nc.dram_tensor("wq", [D, DL], BF16, kind="ExternalInput").ap()
    wk = nc.dram_tensor("wk", [D, DL], BF16, kind="ExternalInput").ap()
    wv = nc.dram_tensor("wv", [D, DL], BF16, kind="ExternalInput").ap()
    bq = nc.dram_tensor("bq", [DL], F32, kind="ExternalInput").ap()
    bk = nc.dram_tensor("bk", [DL], F32, kind="ExternalInput").ap()
    out = nc.dram_tensor("out", [HL * (DH + 1), S], F32, kind="ExternalOutput").ap()

    with tile.TileContext(nc) as tc:
        _build_body(tc, nc, qT, kT, vT, maskT, wq, wk, wv, bq, bk, out)

    nc.compile()
    return nc


def _build_body(tc, nc, qT, kT, vT, maskT, wq, wk, wv, bq, bk, out):
    from contextlib import ExitStack

    with ExitStack() as stk:
        const = stk.enter_context(tc.tile_pool(name="const", bufs=1))
        acts = stk.enter_context(tc.tile_pool(name="acts", bufs=1))
        vt_pool = stk.enter_context(tc.tile_pool(name="vtp", bufs=3))
        e_pool = stk.enter_context(tc.tile_pool(name="epool", bufs=8))
        m_pool = stk.enter_context(tc.tile_pool(name="mpool", bufs=8))
        o_pool = stk.enter_context(tc.tile_pool(name="opool", bufs=2))
        sc_pool = stk.enter_context(tc.tile_pool(name="scp", bufs=2, space="PSUM"))
        cx_pool = stk.enter_context(tc.tile_pool(name="cxp", bufs=1, space="PSUM"))
        pj_pool = stk.enter_context(tc.tile_pool(name="pjp", bufs=2, space="PSUM"))

        # ---- residents ----
        wq_sb = const.tile([128, KT, DL], BF16)
        wk_sb = const.tile([128, KT, DL], BF16)
        wv_sb = const.tile([128, KT, DL], BF16)
        bq_sb = const.tile([128, MT], F32)
        bk_sb = const.tile([128, MT], F32)
        zero_b = const.tile([128, 1], F32)
        kT_sb = const.tile([128, KT, S], BF16)
        qT_sb = const.tile([128, KT, S], BF16)
        qlT_sb = acts.tile([128, MT, S], BF16)   # [d' partition, m-tile, s]
        klT_sb = acts.tile([128, MT, S], BF16)
        vl_sb = acts.tile([128, JT, HL, DH + 1], BF16)  # per j-tile, per head, +ones

        # sync HWDGE queue: weights/biases first, then mask tiles (in loop)
        nc.sync.dma_start(out=wk_sb[:], in_=wk.rearrange("(kt p) n -> p kt n", p=128))
        nc.sync.dma_start(out=wq_sb[:], in_=wq.rearrange("(kt p) n -> p kt n", p=128))
        nc.sync.dma_start(out=wv_sb[:], in_=wv.rearrange("(kt p) n -> p kt n", p=128))
        nc.sync.dma_start(out=bk_sb[:], in_=bk.rearrange("(m p) -> p m", p=128))
        nc.sync.dma_start(out=bq_sb[:], in_=bq.rearrange("(m p) -> p m", p=128))

        # gpsimd SWDGE queue: bulk input chunks, s-block at a time
        vt_tiles = {}

        def load_chunk(dst, src, sb):
            ssl = slice(sb * 512, (sb + 1) * 512)
            nc.gpsimd.dma_start(
                out=dst[:, :, ssl],
                in_=src[:, ssl].rearrange("(kt p) n -> p kt n", p=128))

        def load_vt(sb):
            t = vt_pool.tile([128, KT, 512], BF16, tag="vt")
            ssl = slice(sb * 512, (sb + 1) * 512)
            nc.gpsimd.dma_start(
                out=t[:], in_=vT[:, ssl].rearrange("(kt p) n -> p kt n", p=128))
            vt_tiles[sb] = t

        load_chunk(kT_sb, kT, 0)
        load_chunk(qT_sb, qT, 0)
        load_vt(0)
        load_chunk(kT_sb, kT, 1)
        load_chunk(qT_sb, qT, 1)
        load_vt(1)
        load_chunk(kT_sb, kT, 2)
        load_chunk(qT_sb, qT, 2)
        load_chunk(kT_sb, kT, 3)
        load_chunk(qT_sb, qT, 3)
        load_vt(2)
        load_vt(3)

        nc.vector.memset(zero_b[:], 0.0)
        nc.vector.memset(vl_sb[:, :, :, DH], 1.0)

        # ---- projection chains (tensor-engine filler work) ----
        def k_chain(m, sb):
            ssl = slice(sb * 512, (sb + 1) * 512)
            msl = slice(m * 128, (m + 1) * 128)
            ps = pj_pool.tile([128, 512], F32, tag="pp")
            for kk in range(KT):
                nc.tensor.matmul(ps[:], wk_sb[:, kk, msl], kT_sb[:, kk, ssl],
                                 start=(kk == 0), stop=(kk == KT - 1))
            nc.vector.tensor_scalar_add(klT_sb[:, m, ssl], ps[:], bk_sb[:, m:m + 1])

        def q_chain(m, sb):
            ssl = slice(sb * 512, (sb + 1) * 512)
            msl = slice(m * 128, (m + 1) * 128)
            ps = pj_pool.tile([128, 512], F32, tag="pp")
            for kk in range(KT):
                nc.tensor.matmul(ps[:], wq_sb[:, kk, msl], qT_sb[:, kk, ssl],
                                 start=(kk == 0), stop=(kk == KT - 1))
            nc.vector.tensor_scalar_add(qlT_sb[:, m, ssl], ps[:], bq_sb[:, m:m + 1])

        def v_chain(jt):
            sb, jj = divmod(jt, 4)
            jsl = slice(jj * 128, (jj + 1) * 128)
            ps = pj_pool.tile([128, 512], F32, tag="pp")
            vt = vt_tiles[sb]
            for kk in range(KT):
                nc.tensor.matmul(ps[:], vt[:, kk, jsl], wv_sb[:, kk, :],
                                 start=(kk == 0), stop=(kk == KT - 1))
            nc.vector.tensor_copy(
                vl_sb[:, jt, :, 0:DH], ps[:].rearrange("p (h d) -> p h d", h=HL))

        # static filler schedule: iteration -> projection chains due soon after
        sched = {}

        def at(t, *items):
            sched.setdefault(t, []).extend(items)

        for i in range(JT):
            at(i + 1, ("V", i, 0))          # V(jt) due at iteration jt+LAG
        at(1, ("K", 0, 1))
        at(4, ("K", 0, 2))
        at(7, ("K", 0, 3))
        at(8, ("Q", 0, 1))
        at(17, ("Q", 0, 2))
        at(18, ("K", 1, 0))
        at(22, ("K", 1, 1))
        at(26, ("K", 1, 2))
        at(30, ("K", 1, 3))
        at(33, ("Q", 0, 3))
        at(36, ("Q", 1, 0))
        at(40, ("Q", 1, 1))
        at(44, ("K", 2, 0))
        at(48, ("K", 2, 1))
        at(52, ("K", 2, 2))
        at(56, ("K", 2, 3))
        at(60, ("Q", 1, 2))
        at(64, ("Q", 1, 3))
        at(68, ("K", 3, 0))
        at(72, ("K", 3, 1))
        at(76, ("K", 3, 2))
        at(80, ("K", 3, 3))
        at(84, ("Q", 2, 0))
        at(88, ("Q", 2, 1))
        at(92, ("Q", 2, 2))
        at(96, ("Q", 2, 3))
        at(100, ("Q", 3, 0))
        at(104, ("Q", 3, 1))
        at(108, ("Q", 3, 2))
        at(112, ("Q", 3, 3))

        # lead-in: first K/Q chains so scores(0) can issue immediately
        k_chain(0, 0)
        q_chain(0, 0)

        # ---- fused attention loop ----
        ATT = [(hp, ib, jt) for hp in range(4) for ib in range(4)
               for jt in range(JT)]
        pend = []
        cur_cx = [None]

        def emit_ctx(hp, ib, jt, E):
            h0, h1 = 2 * hp, 2 * hp + 1
            if jt == 0:
                cur_cx[0] = cx_pool.tile([DH + 1, 1024], F32, tag="cx", name="cx")
            cx = cur_cx[0]
            nc.tensor.matmul(cx[:, 0:512], vl_sb[:, jt, h0, :], E[:, 0:512],
                             start=(jt == 0), stop=(jt == JT - 1))
            nc.tensor.matmul(cx[:, 512:1024], vl_sb[:, jt, h1, :], E[:, 512:1024],
                             start=(jt == 0), stop=(jt == JT - 1))
            if jt == JT - 1:
                isl = slice(ib * 512, (ib + 1) * 512)
                o = o_pool.tile([DH + 1, 1024], F32, tag="o")
                nc.vector.tensor_copy(o[:], cx[:])
                nc.gpsimd.dma_start(
                    out=out[(DH + 1) * h0:(DH + 1) * (h0 + 1), isl],
                    in_=o[:, 0:512])
                nc.gpsimd.dma_start(
                    out=out[(DH + 1) * h1:(DH + 1) * (h1 + 1), isl],
                    in_=o[:, 512:1024])

        for t, (hp, ib, jt) in enumerate(ATT):
            isl = slice(ib * 512, (ib + 1) * 512)
            jsl = slice(jt * 128, (jt + 1) * 128)
            msk = m_pool.tile([128, 512], BF16, tag="msk")
            nc.sync.dma_start(out=msk[:], in_=maskT[jsl, isl])
            sc = sc_pool.tile([128, 1024], F32, tag="sc")
            nc.tensor.matmul(
                sc[:, 0:512], klT_sb[0:64, hp, jsl], qlT_sb[0:64, hp, isl],
                start=True, stop=True)
            nc.tensor.matmul(
                sc[:, 512:1024], klT_sb[64:128, hp, jsl], qlT_sb[64:128, hp, isl],
                start=True, stop=True)
            E = e_pool.tile([128, 1024], BF16, tag="E")
            nc.scalar.activation(
                E[:], sc[:], mybir.ActivationFunctionType.Exp,
                bias=zero_b[:], scale=0.125)
            ev = E[:].rearrange("p (o n) -> p o n", o=2)
            mb = msk[:].rearrange("p (o n) -> p o n", o=1)
            mb = mb.broadcast_to([128, 2, 512])
            nc.vector.tensor_tensor(ev, ev, mb, mybir.AluOpType.mult)
            for item in sched.get(t, ()):
                kind, a, b = item
                if kind == "V":
                    v_chain(a)
                elif kind == "K":
                    k_chain(a, b)
                else:
                    q_chain(a, b)
            pend.append((hp, ib, jt, E))
            if len(pend) > LAG:
                emit_ctx(*pend.pop(0))
        while pend:
            emit_ctx(*pend.pop(0))


def _get_graph():
    global _GRAPH
    if _GRAPH is None:
        _GRAPH = build_graph()
    return _GRAPH


def make_in_maps(q, k, v, attention_mask, wq_kernel, wq_bias, wk_kernel,
                 wk_bias, wv_kernel, wv_bias):
    bf = ml_dtypes.bfloat16
    in_maps = []
    for c in range(8):
        b, hg = divmod(c, 2)
        sl = slice(hg * DL, (hg + 1) * DL)
        in_maps.append({
            "qT": np.asarray(q[b].T, dtype=bf),
            "kT": np.asarray(k[b].T, dtype=bf),
            "vT": np.asarray(v[b].T, dtype=bf),
            "maskT": np.asarray(attention_mask[b].T, dtype=bf),
            "wq": np.asarray(wq_kernel[:, sl], dtype=bf),
            "wk": np.asarray(wk_kernel[:, sl], dtype=bf),
            "wv": np.asarray(wv_kernel[:, sl], dtype=bf),
            "bq": np.asarray(wq_bias[sl], dtype=np.float32),
            "bk": np.asarray(wk_bias[sl], dtype=np.float32),
        })
    return in_maps


def assemble_output(results, wv_bias):
    B = 4
    out_full = np.empty((B, S, D), dtype=np.float32)
    for c in range(8):
        b, hg = divmod(c, 2)
        o = results[c]["out"].reshape(HL, DH + 1, S)   # [head, 65, S]
        ctxUT = o[:, :DH, :]                           # [8, 64, S]
        den = o[:, DH, :]                              # [8, S]
        ctxn = ctxUT / den[:, None, :]
        out_full[b, :, hg * DL:(hg + 1) * DL] = (
            ctxn.transpose(2, 0, 1).reshape(S, DL))
    out_full += np.asarray(wv_bias, dtype=np.float32)[None, None, :]
    return out_full


def kernel(q, k, v, attention_mask, wq_kernel, wq_bias, wk_kernel, wk_bias,
           wv_kernel, wv_bias):
    nc = _get_graph()
    in_maps = make_in_maps(q, k, v, attention_mask, wq_kernel, wq_bias,
                           wk_kernel, wk_bias, wv_kernel, wv_bias)
    res = run_bass_kernel_spmd(nc, in_maps, core_ids=list(range(8)))
    return assemble_output(res.results, wv_bias)


# revision 5
# speedup vs baseline: 1.0987x; 1.0442x over previous
"""Trainium2 Bass kernel for nn_AttentionLayer (B=4, S=2048, H=16, DH=64).

Sharding: 8 cores = 4 batches x 2 head-groups (8 heads each). Each core
computes full attention for its (batch, head-group) shard; no cross-core
communication. The host pre-transposes/casts inputs, and post-normalizes
(softmax denominator division), transposes back, and adds the value bias.

v3: fused schedule, ACT-saturation focused.
- The exp ACTIVATE stream (256 x [128,1024] from PSUM, ~1.04us engine
  time each) is the bottleneck; everything else is scheduled around it.
- QKV projections interleave into the attention loop as tensor filler.
- Context matmuls deferred LAG iterations so early V-projection work
  doesn't starve the ACT feed; E tiles buffer in SBUF meanwhile.
- DMA: critical path (wq/wk m0 slices, biases, kT chunk0, mask stream)
  on the sync HWDGE queue; everything else on the gpsimd SWDGE queue,
  ordered by deadline. Biases come host-reshaped [128,4] (a flat [512]
  partition-strided load generates a pathological 4-byte scatter).
- 8 dummy matmuls at the head warm the PE HAM clock gate during the
  initial DMA window.
- Output in bf16 (error budget allows), one [65,1024] copy per (hp,ib).
"""

import numpy as np
import ml_dtypes

import concourse.bass as bass
import concourse.mybir as mybir
import concourse.tile as tile
from concourse import bacc
from concourse.bass_utils import run_bass_kernel_spmd

BF16 = mybir.dt.bfloat16
F32 = mybir.dt.float32

S = 2048      # sequence length
D = 1024      # model dim
DL = 512      # local d' (8 heads x 64)
DH = 64       # head dim
HL = 8        # local heads
KT = 8        # k-tiles over D
MT = 4        # m-tiles over DL (128 each)
JT = 16       # j tiles of 128
LAG = 8       # ctx-matmul deferral (iterations)

_GRAPH = None


def build_graph():
    nc = bacc.Bacc("TRN2", target_bir_lowering=False, debug=False)

    qT = nc.dram_tensor("qT", [D, S], BF16, kind="ExternalInput").ap()
    kT = nc.dram_tensor("kT", [D, S], BF16, kind="ExternalInput").ap()
    vT = nc.dram_tensor("vT", [D, S], BF16, kind="ExternalInput").ap()
    maskT = nc.dram_tensor("maskT", [S, S], BF16, kind="ExternalInput").ap()
    wq = nc.dram_tensor("wq", [D, DL], BF16, kind="ExternalInput").ap()
    wk = nc.dram_tensor("wk", [D, DL], BF16, kind="ExternalInput").ap()
    wv = nc.dram_tensor("wv", [D, DL], BF16, kind="ExternalInput").ap()
    bq = nc.dram_tensor("bq", [128, MT], F32, kind="ExternalInput").ap()
    bk = nc.dram_tensor("bk", [128, MT], F32, kind="ExternalInput").ap()
    out = nc.dram_tensor("out", [HL * (DH + 1), S], BF16, kind="ExternalOutput").ap()

    with tile.TileContext(nc) as tc:
        _build_body(tc, nc, qT, kT, vT, maskT, wq, wk, wv, bq, bk, out)

    nc.compile()
    return nc


def _build_body(tc, nc, qT, kT, vT, maskT, wq, wk, wv, bq, bk, out):
    from contextlib import ExitStack

    with ExitStack() as stk:
        const = stk.enter_context(tc.tile_pool(name="const", bufs=1))
        acts = stk.enter_context(tc.tile_pool(name="acts", bufs=1))
        vt_pool = stk.enter_context(tc.tile_pool(name="vtp", bufs=3))
        e_pool = stk.enter_context(tc.tile_pool(name="epool", bufs=11))
        m_pool = stk.enter_context(tc.tile_pool(name="mpool", bufs=10))
        o_pool = stk.enter_context(tc.tile_pool(name="opool", bufs=2))
        sc_pool = stk.enter_context(tc.tile_pool(name="scp", bufs=2, space="PSUM"))
        cx_pool = stk.enter_context(tc.tile_pool(name="cxp", bufs=1, space="PSUM"))
        pj_pool = stk.enter_context(tc.tile_pool(name="pjp", bufs=2, space="PSUM"))

        # ---- residents ----
        wq0_sb = const.tile([128, KT, 128], BF16)   # m-tile 0 slice (critical path)
        wk0_sb = const.tile([128, KT, 128], BF16)
        wqr_sb = const.tile([128, KT, 384], BF16)   # m-tiles 1..3
        wkr_sb = const.tile([128, KT, 384], BF16)
        wv_sb = const.tile([128, KT, DL], BF16)
        bq_sb = const.tile([128, MT], F32)
        bk_sb = const.tile([128, MT], F32)
        zero_b = const.tile([128, 1], F32)
        kT_sb = const.tile([128, KT, S], BF16)
        qT_sb = const.tile([128, KT, S], BF16)
        qlT_sb = acts.tile([128, MT, S], BF16)   # [d' partition, m-tile, s]
        klT_sb = acts.tile([128, MT, S], BF16)
        vl_sb = acts.tile([128, JT, HL, DH + 1], BF16)  # per j-tile, per head, +ones

        def chunk(src, sb):
            ssl = slice(sb * 512, (sb + 1) * 512)
            return src[:, ssl].rearrange("(kt p) n -> p kt n", p=128)

        # sync HWDGE queue: critical path first, then the mask stream
        nc.sync.dma_start(out=wq0_sb[:], in_=wq[:, 0:128].rearrange(
            "(kt p) n -> p kt n", p=128))
        nc.sync.dma_start(out=wk0_sb[:], in_=wk[:, 0:128].rearrange(
            "(kt p) n -> p kt n", p=128))
        nc.sync.dma_start(out=bk_sb[:], in_=bk)
        nc.sync.dma_start(out=bq_sb[:], in_=bq)
        nc.sync.dma_start(out=kT_sb[:, :, 0:512], in_=chunk(kT, 0))

        # gpsimd SWDGE queue: ordered by deadline
        vt_tiles = {}

        def load_vt(sb):
            t = vt_pool.tile([128, KT, 512], BF16, tag="vt", name="vt")
            nc.gpsimd.dma_start(out=t[:], in_=chunk(vT, sb))
            vt_tiles[sb] = t

        nc.gpsimd.dma_start(out=qT_sb[:, :, 0:512], in_=chunk(qT, 0))
        nc.gpsimd.dma_start(out=kT_sb[:, :, 512:1024], in_=chunk(kT, 1))
        nc.gpsimd.dma_start(out=kT_sb[:, :, 1024:1536], in_=chunk(kT, 2))
        nc.gpsimd.dma_start(out=wv_sb[:], in_=wv.rearrange(
            "(kt p) n -> p kt n", p=128))
        nc.gpsimd.dma_start(out=kT_sb[:, :, 1536:2048], in_=chunk(kT, 3))
        load_vt(0)
        nc.gpsimd.dma_start(out=wqr_sb[:], in_=wq[:, 128:512].rearrange(
            "(kt p) n -> p kt n", p=128))
        nc.gpsimd.dma_start(out=wkr_sb[:], in_=wk[:, 128:512].rearrange(
            "(kt p) n -> p kt n", p=128))
        nc.gpsimd.dma_start(out=qT_sb[:, :, 512:1024], in_=chunk(qT, 1))
        load_vt(1)
        nc.gpsimd.dma_start(out=qT_sb[:, :, 1024:1536], in_=chunk(qT, 2))
        nc.gpsimd.dma_start(out=qT_sb[:, :, 1536:2048], in_=chunk(qT, 3))
        load_vt(2)
        load_vt(3)

        nc.vector.memset(zero_b[:], 0.0)
        nc.vector.memset(vl_sb[:, :, :, DH], 1.0)

        # ---- PE HAM warmup: dummy matmuls during the initial DMA window ----
        for w in range(8):
            wt = cx_pool.tile([128, 512], F32, tag="cx", name="warm")
            nc.tensor.matmul(wt[:], qlT_sb[:, 0, 0:128], qlT_sb[:, 0, 0:512],
                             start=True, stop=True)

        # ---- projection chains (tensor-engine filler work) ----
        def wslice(m, w0, wr):
            if m == 0:
                return (w0, slice(0, 128))
            return (wr, slice((m - 1) * 128, m * 128))

        def k_chain(m, sb):
            ssl = slice(sb * 512, (sb + 1) * 512)
            wsb, msl = wslice(m, wk0_sb, wkr_sb)
            ps = pj_pool.tile([128, 512], F32, tag="pp", name="psk")
            for kk in range(KT):
                nc.tensor.matmul(ps[:], wsb[:, kk, msl], kT_sb[:, kk, ssl],
                                 start=(kk == 0), stop=(kk == KT - 1))
            nc.vector.tensor_scalar_add(klT_sb[:, m, ssl], ps[:], bk_sb[:, m:m + 1])

        def q_chain(m, sb):
            ssl = slice(sb * 512, (sb + 1) * 512)
            wsb, msl = wslice(m, wq0_sb, wqr_sb)
            ps = pj_pool.tile([128, 512], F32, tag="pp", name="psq")
            for kk in range(KT):
                nc.tensor.matmul(ps[:], wsb[:, kk, msl], qT_sb[:, kk, ssl],
                                 start=(kk == 0), stop=(kk == KT - 1))
            nc.vector.tensor_scalar_add(qlT_sb[:, m, ssl], ps[:], bq_sb[:, m:m + 1])

        def v_chain(jt):
            sb, jj = divmod(jt, 4)
            jsl = slice(jj * 128, (jj + 1) * 128)
            ps = pj_pool.tile([128, 512], F32, tag="pp", name="psv")
            vt = vt_tiles[sb]
            for kk in range(KT):
                nc.tensor.matmul(ps[:], vt[:, kk, jsl], wv_sb[:, kk, :],
                                 start=(kk == 0), stop=(kk == KT - 1))
            nc.vector.tensor_copy(
                vl_sb[:, jt, :, 0:DH], ps[:].rearrange("p (h d) -> p h d", h=HL))

        # static filler schedule: iteration -> projection chains due soon after
        sched = {}

        def at(t, *items):
            sched.setdefault(t, []).extend(items)

        for i in range(JT):
            at(i + 6, ("V", i, 0))          # V(jt) due at iteration jt+LAG
        at(2, ("K", 0, 1))
        at(5, ("K", 0, 2))
        at(8, ("K", 0, 3))
        at(10, ("Q", 0, 1))
        at(23, ("Q", 0, 2))
        at(25, ("K", 1, 0))
        at(28, ("K", 1, 1))
        at(31, ("K", 1, 2))
        at(34, ("K", 1, 3))
        at(37, ("Q", 0, 3))
        at(40, ("Q", 1, 0))
        at(44, ("Q", 1, 1))
        at(48, ("K", 2, 0))
        at(52, ("K", 2, 1))
        at(56, ("K", 2, 2))
        at(60, ("K", 2, 3))
        at(64, ("Q", 1, 2))
        at(68, ("Q", 1, 3))
        at(72, ("K", 3, 0))
        at(76, ("K", 3, 1))
        at(80, ("K", 3, 2))
        at(84, ("K", 3, 3))
        at(88, ("Q", 2, 0))
        at(92, ("Q", 2, 1))
        at(96, ("Q", 2, 2))
        at(100, ("Q", 2, 3))
        at(104, ("Q", 3, 0))
        at(108, ("Q", 3, 1))
        at(112, ("Q", 3, 2))
        at(116, ("Q", 3, 3))

        # lead-in projections for (hp=0, ib=0, jt=0)
        q_chain(0, 0)
        k_chain(0, 0)

        # ---- fused attention loop ----
        ATT = [(hp, ib, jt) for hp in range(4) for ib in range(4)
               for jt in range(JT)]
        NIT = len(ATT)
        pend = []
        cur_cx = [None]

        def emit_ctx(hp, ib, jt, E):
            h0, h1 = 2 * hp, 2 * hp + 1
            if jt == 0:
                cur_cx[0] = cx_pool.tile([DH + 1, 1024], F32, tag="cx", name="cx")
            cx = cur_cx[0]
            nc.tensor.matmul(cx[:, 0:512], vl_sb[:, jt, h0, :], E[:, 0:512],
                             start=(jt == 0), stop=(jt == JT - 1))
            nc.tensor.matmul(cx[:, 512:1024], vl_sb[:, jt, h1, :], E[:, 512:1024],
                             start=(jt == 0), stop=(jt == JT - 1))
            if jt == JT - 1:
                isl = slice(ib * 512, (ib + 1) * 512)
                o = o_pool.tile([DH + 1, 1024], BF16, tag="o", name="o")
                nc.vector.tensor_copy(o[:], cx[:])
                nc.gpsimd.dma_start(
                    out=out[(DH + 1) * h0:(DH + 1) * (h0 + 1), isl],
                    in_=o[:, 0:512])
                nc.gpsimd.dma_start(
                    out=out[(DH + 1) * h1:(DH + 1) * (h1 + 1), isl],
                    in_=o[:, 512:1024])

        for t, (hp, ib, jt) in enumerate(ATT):
            isl = slice(ib * 512, (ib + 1) * 512)
            jsl = slice(jt * 128, (jt + 1) * 128)
            msk = m_pool.tile([128, 512], BF16, tag="msk", name="msk")
            nc.sync.dma_start(out=msk[:], in_=maskT[jsl, isl])
            sc = sc_pool.tile([128, 1024], F32, tag="sc", name="sc")
            nc.tensor.matmul(
                sc[:, 0:512], klT_sb[0:64, hp, jsl], qlT_sb[0:64, hp, isl],
                start=True, stop=True)
            nc.tensor.matmul(
                sc[:, 512:1024], klT_sb[64:128, hp, jsl], qlT_sb[64:128, hp, isl],
                start=True, stop=True)
            E = e_pool.tile([128, 1024], BF16, tag="E", name="E")
            nc.scalar.activation(
                E[:], sc[:], mybir.ActivationFunctionType.Exp,
                bias=zero_b[:], scale=0.125)
            ev = E[:].rearrange("p (o n) -> p o n", o=2)
            mb = msk[:].rearrange("p (o n) -> p o n", o=1)
            mb = mb.broadcast_to([128, 2, 512])
            nc.vector.tensor_tensor(ev, ev, mb, mybir.AluOpType.mult)
            for item in sched.get(t, ()):
                kind, a, b = item
                if kind == "V":
                    v_chain(a)
                elif kind == "K":
                    k_chain(a, b)
                else:
                    q_chain(a, b)
            pend.append((hp, ib, jt, E))
            npop = 1 if len(pend) > LAG else 0
            if t >= NIT - LAG:       # tail taper: drain 2/iter at the end
                npop = 2
            for _ in range(min(npop, len(pend))):
                emit_ctx(*pend.pop(0))
        while pend:
            emit_ctx(*pend.pop(0))


def _get_graph():
    global _GRAPH
    if _GRAPH is None:
        _GRAPH = build_graph()
    return _GRAPH


def make_in_maps(q, k, v, attention_mask, wq_kernel, wq_bias, wk_kernel,
                 wk_bias, wv_kernel, wv_bias):
    bf = ml_dtypes.bfloat16
    in_maps = []
    for c in range(8):
        b, hg = divmod(c, 2)
        sl = slice(hg * DL, (hg + 1) * DL)
        in_maps.append({
            "qT": np.asarray(q[b].T, dtype=bf),
            "kT": np.asarray(k[b].T, dtype=bf),
            "vT": np.asarray(v[b].T, dtype=bf),
            "maskT": np.asarray(attention_mask[b].T, dtype=bf),
            "wq": np.asarray(wq_kernel[:, sl], dtype=bf),
            "wk": np.asarray(wk_kernel[:, sl], dtype=bf),
            "wv": np.asarray(wv_kernel[:, sl], dtype=bf),
            "bq": np.ascontiguousarray(
                np.asarray(wq_bias[sl], dtype=np.float32).reshape(MT, 128).T),
            "bk": np.ascontiguousarray(
                np.asarray(wk_bias[sl], dtype=np.float32).reshape(MT, 128).T),
        })
    return in_maps


def assemble_output(results, wv_bias):
    B = 4
    out_full = np.empty((B, S, D), dtype=np.float32)
    for c in range(8):
        b, hg = divmod(c, 2)
        o = np.asarray(results[c]["out"], dtype=np.float32)
        o = o.reshape(HL, DH + 1, S)                   # [head, 65, S]
        ctxUT = o[:, :DH, :]                           # [8, 64, S]
        den = o[:, DH, :]                              # [8, S]
        ctxn = ctxUT / den[:, None, :]
        out_full[b, :, hg * DL:(hg + 1) * DL] = (
            ctxn.transpose(2, 0, 1).reshape(S, DL))
    out_full += np.asarray(wv_bias, dtype=np.float32)[None, None, :]
    return out_full


def kernel(q, k, v, attention_mask, wq_kernel, wq_bias, wk_kernel, wk_bias,
           wv_kernel, wv_bias):
    nc = _get_graph()
    in_maps = make_in_maps(q, k, v, attention_mask, wq_kernel, wq_bias,
                           wk_kernel, wk_bias, wv_kernel, wv_bias)
    res = run_bass_kernel_spmd(nc, in_maps, core_ids=list(range(8)))
    return assemble_output(res.results, wv_bias)


# revision 8
# speedup vs baseline: 1.1489x; 1.0457x over previous
"""Trainium2 Bass kernel for nn_AttentionLayer (B=4, S=2048, H=16, DH=64).

Sharding: 8 cores = 4 batches x 2 head-groups (8 heads each). Each core
computes full attention for its (batch, head-group) shard; no cross-core
communication. The host pre-transposes/casts inputs, and post-normalizes
(softmax denominator division), transposes back, and adds the value bias.

v3: fused schedule, ACT-saturation focused.
- The exp ACTIVATE stream (256 x [128,1024] from PSUM, ~1.04us engine
  time each) is the bottleneck; everything else is scheduled around it.
- QKV projections interleave into the attention loop as tensor filler.
- Context matmuls deferred LAG iterations so early V-projection work
  doesn't starve the ACT feed; E tiles buffer in SBUF meanwhile.
- DMA: critical path (wq/wk m0 slices, biases, kT chunk0, mask stream)
  on the sync HWDGE queue; everything else on the gpsimd SWDGE queue,
  ordered by deadline. Biases come host-reshaped [128,4] (a flat [512]
  partition-strided load generates a pathological 4-byte scatter).
- 8 dummy matmuls at the head warm the PE HAM clock gate during the
  initial DMA window.
- Output in bf16 (error budget allows), one [65,1024] copy per (hp,ib).
"""

import numpy as np
import ml_dtypes

import concourse.bass as bass
import concourse.mybir as mybir
import concourse.tile as tile
from concourse import bacc
from concourse.bass_utils import run_bass_kernel_spmd

BF16 = mybir.dt.bfloat16
F32 = mybir.dt.float32

S = 2048      # sequence length
D = 1024      # model dim
DL = 512      # local d' (8 heads x 64)
DH = 64       # head dim
HL = 8        # local heads
KT = 8        # k-tiles over D
MT = 4        # m-tiles over DL (128 each)
JT = 16       # j tiles of 128
LAG = 8       # ctx-matmul deferral (iterations)

_GRAPH = None


def build_graph():
    nc = bacc.Bacc("TRN2", target_bir_lowering=False, debug=False)

    qT = nc.dram_tensor("qT", [D, S], BF16, kind="ExternalInput").ap()
    kT = nc.dram_tensor("kT", [D, S], BF16, kind="ExternalInput").ap()
    vT = nc.dram_tensor("vT", [D, S], BF16, kind="ExternalInput").ap()
    maskT = nc.dram_tensor("maskT", [S, S], BF16, kind="ExternalInput").ap()
    wq = nc.dram_tensor("wq", [D, DL], BF16, kind="ExternalInput").ap()
    wk = nc.dram_tensor("wk", [D, DL], BF16, kind="ExternalInput").ap()
    wv = nc.dram_tensor("wv", [D, DL], BF16, kind="ExternalInput").ap()
    bq = nc.dram_tensor("bq", [128, MT], F32, kind="ExternalInput").ap()
    bk = nc.dram_tensor("bk", [128, MT], F32, kind="ExternalInput").ap()
    out = nc.dram_tensor("out", [HL * (DH + 1), S], BF16, kind="ExternalOutput").ap()

    with tile.TileContext(nc) as tc:
        _build_body(tc, nc, qT, kT, vT, maskT, wq, wk, wv, bq, bk, out)

    nc.compile()
    return nc


def _build_body(tc, nc, qT, kT, vT, maskT, wq, wk, wv, bq, bk, out):
    from contextlib import ExitStack

    with ExitStack() as stk:
        const = stk.enter_context(tc.tile_pool(name="const", bufs=1))
        acts = stk.enter_context(tc.tile_pool(name="acts", bufs=1))
        vt_pool = stk.enter_context(tc.tile_pool(name="vtp", bufs=3))
        e_pool = stk.enter_context(tc.tile_pool(name="epool", bufs=11))
        m_pool = stk.enter_context(tc.tile_pool(name="mpool", bufs=10))
        o_pool = stk.enter_context(tc.tile_pool(name="opool", bufs=2))
        sc_pool = stk.enter_context(tc.tile_pool(name="scp", bufs=2, space="PSUM"))
        cx_pool = stk.enter_context(tc.tile_pool(name="cxp", bufs=1, space="PSUM"))
        pj_pool = stk.enter_context(tc.tile_pool(name="pjp", bufs=2, space="PSUM"))

        # ---- residents ----
        wq0_sb = const.tile([128, KT, 128], BF16)   # m-tile 0 slice (critical path)
        wk0_sb = const.tile([128, KT, 128], BF16)
        wqr_sb = const.tile([128, KT, 384], BF16)   # m-tiles 1..3
        wkr_sb = const.tile([128, KT, 384], BF16)
        wv_sb = const.tile([128, KT, DL], BF16)
        bq_sb = const.tile([128, MT], F32)
        bk_sb = const.tile([128, MT], F32)
        zero_b = const.tile([128, 1], F32)
        kT_sb = const.tile([128, KT, S], BF16)
        qT_sb = const.tile([128, KT, S], BF16)
        qlT_sb = acts.tile([128, MT, S], BF16)   # [d' partition, m-tile, s]
        klT_sb = acts.tile([128, MT, S], BF16)
        vl_sb = acts.tile([128, JT, HL, DH + 1], BF16)  # per j-tile, per head, +ones

        gate_sb = const.tile([1, 8], BF16)

        def chunk(src, sb):
            ssl = slice(sb * 512, (sb + 1) * 512)
            return src[:, ssl].rearrange("(kt p) n -> p kt n", p=128)

        # Critical-path DMAs only; everything else is gated behind
        # iteration 0 (see emit_deferred_loads) to keep HBM free for these.
        # sync HWDGE queue: Q-path + weights, then the mask stream.
        nc.sync.dma_start(out=wq0_sb[:], in_=wq[:, 0:128].rearrange(
            "(kt p) n -> p kt n", p=128))
        nc.sync.dma_start(out=qT_sb[:, :, 0:512], in_=chunk(qT, 0))
        nc.sync.dma_start(out=wk0_sb[:], in_=wk[:, 0:128].rearrange(
            "(kt p) n -> p kt n", p=128))
        nc.sync.dma_start(out=bq_sb[:], in_=bq)
        nc.sync.dma_start(out=bk_sb[:], in_=bk)
        # gpsimd SWDGE queue: K-path chunks.
        nc.gpsimd.dma_start(out=kT_sb[:, :, 0:512], in_=chunk(kT, 0))
        nc.gpsimd.dma_start(out=kT_sb[:, :, 512:1024], in_=chunk(kT, 1))

        vt_tiles = {}

        def load_vt(sb):
            t = vt_pool.tile([128, KT, 512], BF16, tag="vt", name="vt")
            nc.gpsimd.dma_start(out=t[:], in_=chunk(vT, sb))
            vt_tiles[sb] = t

        def emit_deferred_loads(E0):
            # Gate: a trivial gpsimd op depending on iteration 0's E tile.
            # All bulk loads behind it start only once the pipeline is live,
            # keeping startup HBM bandwidth for the critical path.
            nc.gpsimd.tensor_copy(gate_sb[:], E0[0:1, 0:8])
            nc.gpsimd.dma_start(out=kT_sb[:, :, 1024:1536], in_=chunk(kT, 2))
            nc.gpsimd.dma_start(out=wv_sb[:], in_=wv.rearrange(
                "(kt p) n -> p kt n", p=128))
            load_vt(0)
            nc.gpsimd.dma_start(out=kT_sb[:, :, 1536:2048], in_=chunk(kT, 3))
            nc.gpsimd.dma_start(out=qT_sb[:, :, 512:1024], in_=chunk(qT, 1))
            nc.gpsimd.dma_start(out=wqr_sb[:], in_=wq[:, 128:512].rearrange(
                "(kt p) n -> p kt n", p=128))
            load_vt(1)
            nc.gpsimd.dma_start(out=wkr_sb[:], in_=wk[:, 128:512].rearrange(
                "(kt p) n -> p kt n", p=128))
            nc.gpsimd.dma_start(out=qT_sb[:, :, 1024:1536], in_=chunk(qT, 2))
            nc.gpsimd.dma_start(out=qT_sb[:, :, 1536:2048], in_=chunk(qT, 3))
            load_vt(2)
            load_vt(3)

        nc.vector.memset(zero_b[:], 0.0)
        nc.vector.memset(vl_sb[:, :, :, DH], 1.0)

        # ---- PE HAM warmup: dummy matmuls during the initial DMA window ----
        for w in range(8):
            wt = cx_pool.tile([128, 512], F32, tag="cx", name="warm")
            nc.tensor.matmul(wt[:], qlT_sb[:, 0, 0:128], qlT_sb[:, 0, 0:512],
                             start=True, stop=True)

        # ---- projection chains (tensor-engine filler work) ----
        # Each chain = 8 accumulating matmuls + an epilogue; emitted in two
        # 4-matmul slices on consecutive iterations so a whole chain never
        # sits between two scores matmuls in the tensor FIFO.
        def wslice(m, w0, wr):
            if m == 0:
                return (w0, slice(0, 128))
            return (wr, slice((m - 1) * 128, m * 128))

        def chain_mms(kind, a, b, ps, lo, hi):
            if kind == "V":
                sb, jj = divmod(a, 4)
                jsl = slice(jj * 128, (jj + 1) * 128)
                vt = vt_tiles[sb]
                for kk in range(lo, hi):
                    nc.tensor.matmul(ps[:], vt[:, kk, jsl], wv_sb[:, kk, :],
                                     start=(kk == 0), stop=(kk == KT - 1))
            else:
                w0, wr = (wq0_sb, wqr_sb) if kind == "Q" else (wk0_sb, wkr_sb)
                src = qT_sb if kind == "Q" else kT_sb
                wsb, msl = wslice(a, w0, wr)
                ssl = slice(b * 512, (b + 1) * 512)
                for kk in range(lo, hi):
                    nc.tensor.matmul(ps[:], wsb[:, kk, msl], src[:, kk, ssl],
                                     start=(kk == 0), stop=(kk == KT - 1))

        def chain_epilogue(kind, a, b, ps):
            if kind == "V":
                nc.vector.tensor_copy(
                    vl_sb[:, a, :, 0:DH],
                    ps[:].rearrange("p (h d) -> p h d", h=HL))
            elif kind == "Q":
                ssl = slice(b * 512, (b + 1) * 512)
                nc.vector.tensor_scalar_add(
                    qlT_sb[:, a, ssl], ps[:], bq_sb[:, a:a + 1])
            else:
                ssl = slice(b * 512, (b + 1) * 512)
                nc.vector.tensor_scalar_add(
                    klT_sb[:, a, ssl], ps[:], bk_sb[:, a:a + 1])

        def full_chain(kind, a, b):
            ps = pj_pool.tile([128, 512], F32, tag="pp", name="pp")
            chain_mms(kind, a, b, ps, 0, KT)
            chain_epilogue(kind, a, b, ps)

        # static filler schedule: iteration -> projection chains due soon after
        sched = {}

        def at(t, *items):
            sched.setdefault(t, []).extend(items)

        for i in range(JT):
            at(i + 5, ("V", i, 0))          # V(jt) due at iteration jt+LAG
        at(2, ("K", 0, 1))
        at(5, ("K", 0, 2))
        at(8, ("K", 0, 3))
        at(10, ("Q", 0, 1))
        at(26, ("Q", 0, 2))
        at(42, ("Q", 0, 3))
        at(46, ("K", 1, 0))
        at(50, ("K", 1, 1))
        at(54, ("K", 1, 2))
        at(58, ("K", 1, 3))
        at(56, ("Q", 1, 0))
        at(70, ("Q", 1, 1))
        at(86, ("Q", 1, 2))
        at(102, ("Q", 1, 3))
        at(108, ("K", 2, 0))
        at(112, ("K", 2, 1))
        at(116, ("K", 2, 2))
        at(120, ("K", 2, 3))
        at(124, ("Q", 2, 0))
        at(134, ("Q", 2, 1))
        at(150, ("Q", 2, 2))
        at(166, ("Q", 2, 3))
        at(172, ("K", 3, 0))
        at(176, ("K", 3, 1))
        at(180, ("K", 3, 2))
        at(184, ("K", 3, 3))
        at(188, ("Q", 3, 0))
        at(198, ("Q", 3, 1))
        at(214, ("Q", 3, 2))
        at(230, ("Q", 3, 3))

        # lead-in projections for (hp=0, ib=0, jt=0)
        full_chain("Q", 0, 0)
        full_chain("K", 0, 0)

        # ---- fused attention loop ----
        ATT = [(hp, ib, jt) for hp in range(4) for ib in range(4)
               for jt in range(JT)]
        NIT = len(ATT)
        pend = []
        cur_cx = [None]

        def emit_ctx(hp, ib, jt, E):
            h0, h1 = 2 * hp, 2 * hp + 1
            if jt == 0:
                cur_cx[0] = cx_pool.tile([DH + 1, 1024], F32, tag="cx", name="cx")
            cx = cur_cx[0]
            nc.tensor.matmul(cx[:, 0:512], vl_sb[:, jt, h0, :], E[:, 0:512],
                             start=(jt == 0), stop=(jt == JT - 1))
            nc.tensor.matmul(cx[:, 512:1024], vl_sb[:, jt, h1, :], E[:, 512:1024],
                             start=(jt == 0), stop=(jt == JT - 1))
            if jt == JT - 1:
                isl = slice(ib * 512, (ib + 1) * 512)
                o = o_pool.tile([DH + 1, 1024], BF16, tag="o", name="o")
                nc.vector.tensor_copy(o[:], cx[:])
                nc.gpsimd.dma_start(
                    out=out[(DH + 1) * h0:(DH + 1) * (h0 + 1), isl],
                    in_=o[:, 0:512])
                nc.gpsimd.dma_start(
                    out=out[(DH + 1) * h1:(DH + 1) * (h1 + 1), isl],
                    in_=o[:, 512:1024])

        second_half = []
        for t, (hp, ib, jt) in enumerate(ATT):
            isl = slice(ib * 512, (ib + 1) * 512)
            jsl = slice(jt * 128, (jt + 1) * 128)
            msk = m_pool.tile([128, 512], BF16, tag="msk", name="msk")
            nc.sync.dma_start(out=msk[:], in_=maskT[jsl, isl])
            sc = sc_pool.tile([128, 1024], F32, tag="sc", name="sc")
            nc.tensor.matmul(
                sc[:, 0:512], klT_sb[0:64, hp, jsl], qlT_sb[0:64, hp, isl],
                start=True, stop=True)
            nc.tensor.matmul(
                sc[:, 512:1024], klT_sb[64:128, hp, jsl], qlT_sb[64:128, hp, isl],
                start=True, stop=True)
            E = e_pool.tile([128, 1024], BF16, tag="E", name="E")
            nc.scalar.activation(
                E[:], sc[:], mybir.ActivationFunctionType.Exp,
                bias=zero_b[:], scale=0.125)
            ev = E[:].rearrange("p (o n) -> p o n", o=2)
            mb = msk[:].rearrange("p (o n) -> p o n", o=1)
            mb = mb.broadcast_to([128, 2, 512])
            nc.vector.tensor_tensor(ev, ev, mb, mybir.AluOpType.mult)
            if t == 1:
                emit_deferred_loads(E0)
            # finish last iteration's chains, then start this iteration's
            for kind, a, b, ps in second_half:
                chain_mms(kind, a, b, ps, 4, KT)
                chain_epilogue(kind, a, b, ps)
            second_half = []
            for item in sched.get(t, ()):
                kind, a, b = item
                ps = pj_pool.tile([128, 512], F32, tag="pp", name="pp")
                chain_mms(kind, a, b, ps, 0, 4)
                second_half.append((kind, a, b, ps))
            if t == 0:
                E0 = E
            pend.append((hp, ib, jt, E))
            npop = 1 if len(pend) > LAG else 0
            if t >= NIT - LAG:       # tail taper: drain 2/iter at the end
                npop = 2
            for _ in range(min(npop, len(pend))):
                emit_ctx(*pend.pop(0))
        while pend:
            emit_ctx(*pend.pop(0))


def _get_graph():
    global _GRAPH
    if _GRAPH is None:
        _GRAPH = build_graph()
    return _GRAPH


def make_in_maps(q, k, v, attention_mask, wq_kernel, wq_bias, wk_kernel,
                 wk_bias, wv_kernel, wv_bias):
    bf = ml_dtypes.bfloat16
    in_maps = []
    for c in range(8):
        b, hg = divmod(c, 2)
        sl = slice(hg * DL, (hg + 1) * DL)
        in_maps.append({
            "qT": np.asarray(q[b].T, dtype=bf),
            "kT": np.asarray(k[b].T, dtype=bf),
            "vT": np.asarray(v[b].T, dtype=bf),
            "maskT": np.asarray(attention_mask[b].T, dtype=bf),
            "wq": np.asarray(wq_kernel[:, sl], dtype=bf),
            "wk": np.asarray(wk_kernel[:, sl], dtype=bf),
            "wv": np.asarray(wv_kernel[:, sl], dtype=bf),
            "bq": np.ascontiguousarray(
                np.asarray(wq_bias[sl], dtype=np.float32).reshape(MT, 128).T),
            "bk": np.ascontiguousarray(
                np.asarray(wk_bias[sl], dtype=np.float32).reshape(MT, 128).T),
        })
    return in_maps


def assemble_output(results, wv_bias):
    B = 4
    out_full = np.empty((B, S, D), dtype=np.float32)
    for c in range(8):
        b, hg = divmod(c, 2)
        o = np.asarray(results[c]["out"], dtype=np.float32)
        o = o.reshape(HL, DH + 1, S)                   # [head, 65, S]
        ctxUT = o[:, :DH, :]                           # [8, 64, S]
        den = o[:, DH, :]                              # [8, S]
        ctxn = ctxUT / den[:, None, :]
        out_full[b, :, hg * DL:(hg + 1) * DL] = (
            ctxn.transpose(2, 0, 1).reshape(S, DL))
    out_full += np.asarray(wv_bias, dtype=np.float32)[None, None, :]
    return out_full


def kernel(q, k, v, attention_mask, wq_kernel, wq_bias, wk_kernel, wk_bias,
           wv_kernel, wv_bias):
    nc = _get_graph()
    in_maps = make_in_maps(q, k, v, attention_mask, wq_kernel, wq_bias,
                           wk_kernel, wk_bias, wv_kernel, wv_bias)
    res = run_bass_kernel_spmd(nc, in_maps, core_ids=list(range(8)))
    return assemble_output(res.results, wv_bias)
